# revision 22
# baseline (speedup 1.0000x reference)
"""Bass/TRN2 kernel for nn_BMM_S8T_S8N_S8T:
    out[b,m,n] = sat_i8(round(alpha * sum_k a[b,m,k] * b[b,n,k]))
with a: (32, 2048, 64) int8, b: (32, 2048, 64) int8, alpha: f32 scalar.

Sharding: batch dim 32 -> 8 cores x 4 batches (pure data parallel, no
cross-core communication).

Per-core design notes:
  - int8 matmul is not supported by the PE; bf16 x bf16 -> f32 PSUM is exact
    for int8 operands (products < 2^16, sums of 64 < 2^24), so inputs are
    converted to bf16 (and pre-transposed to [K, seq] layout) on host.
  - The 4 local batches are stacked in pairs along SBUF partitions:
    partitions 0-63 hold batch 2p's K=64, partitions 64-127 batch 2p+1's.
    Row-tiled matmuls (tile_position row groups 0 and 2) then run the two
    batches' K=64 contractions concurrently on the 128x128 PE array.
  - Requant drain (PSUM f32 -> SBUF int8, scale + round-half-even + saturate)
    is the bottleneck: only VectorE and ScalarE can read PSUM, at ~1 elem/
    lane/cycle. Both engines drain in parallel on different PSUM banks, in
    [128, 1024] (2-bank) units so fills overlap drains within 8 banks.
    A single tensor_scalar_mul / activation(Copy, scale) instruction does the
    whole requant bit-exactly (verified vs numpy round/clip on HW).
"""

import numpy as np
import ml_dtypes

B, M, N, K = 32, 2048, 2048, 64
NCORES = 8
BPC = B // NCORES          # batches per core (4)
MT = M // 128              # m-tiles per batch (16)
NHALF = 2                  # two 1024-col drain units per m-tile row block
UNIT = N // NHALF          # 1024 columns per drain unit

_CACHE = {}


def _build(alpha: float):
    import concourse.bacc as bacc
    import concourse.mybir as mybir
    from concourse.tile import TileContext

    bf16 = mybir.dt.bfloat16
    f32 = mybir.dt.float32
    i8 = mybir.dt.int8

    nc = bacc.Bacc("TRN2")
    aT = nc.dram_tensor("aT", (BPC // 2, 128, M), bf16, kind="ExternalInput")
    bT = nc.dram_tensor("bT", (BPC // 2, 128, N), bf16, kind="ExternalInput")
    out = nc.dram_tensor("out", (BPC, M, N), i8, kind="ExternalOutput")

    # engine load balancing between the two drain engines (ns per [128,1024]
    # unit, hardware-measured)
    DVE_NS, ACT_NS = 1224.0, 1113.0
    WARMUP_MM = 6  # ~3.5us of dummy matmuls to lift the PE HAM clock gate

    # scratch operand for warm-up matmuls: deliberately never written (the
    # values don't matter and the scratch PSUM bank is never read); a raw
    # (non-pool) tensor so Tile's release tracking doesn't object.
    wz = nc.alloc_sbuf_tensor("wz", [128, 512], bf16)

    with TileContext(nc) as tc:
        with (
            tc.tile_pool(name="inp", bufs=1) as inp_pool,
            tc.tile_pool(name="ps", bufs=4, space="PSUM") as psum_pool,
            tc.tile_pool(name="outp", bufs=6) as out_pool,
        ):
            wps = psum_pool.tile([128, UNIT], f32, tag="ps")
            for _ in range(WARMUP_MM):
                nc.tensor.matmul(
                    wps[:, 0:512], wz[:, 0:128], wz[:, 0:512], start=True, stop=True
                )

            # Input tiles are split by "when first needed" — dependencies are
            # tile-granular and each individual DMA transfer only streams at
            # ~90-100 GB/s, so pair-0 inputs are loaded as several concurrent
            # DMAs (across the Sync HWDGE and GpSimd SWDGE queue streams) in
            # order of first use, letting the t=0 matmuls start early while
            # the rest lands behind the already-running pipeline.
            b0_q = [
                inp_pool.tile([128, 512], bf16, tag=f"b0q{q}", name=f"b0q{q}")
                for q in range(4)
            ]
            a0_first = inp_pool.tile([128, 128], bf16, tag="a0f")
            a0_r0 = inp_pool.tile([128, 896], bf16, tag="a0r0")
            a0_r1 = inp_pool.tile([128, 1024], bf16, tag="a0r1")
            a1_sb = inp_pool.tile([128, M], bf16, tag="a1")
            b1_sb = inp_pool.tile([128, N], bf16, tag="b1")
            # completions are FIFO per DGE ring, so the b pieces stream on the
            # Sync HWDGE ring and the a pieces on the GpSimd SWDGE ring, each
            # in order of first use
            for q in range(4):
                nc.sync.dma_start(
                    out=b0_q[q][:, :], in_=bT[0, :, 512 * q : 512 * (q + 1)]
                )
            nc.sync.dma_start(out=b1_sb[:, :], in_=bT[1])
            nc.gpsimd.dma_start(out=a0_first[:, :], in_=aT[0, :, 0:128])
            nc.gpsimd.dma_start(out=a0_r0[:, :], in_=aT[0, :, 128:1024])
            nc.gpsimd.dma_start(out=a0_r1[:, :], in_=aT[0, :, 1024:M])
            nc.gpsimd.dma_start(out=a1_sb[:, :], in_=aT[1])

            def lhs_ap(p, t, rows):
                if p == 1:
                    return a1_sb[rows, 128 * t : 128 * (t + 1)]
                if t == 0:
                    return a0_first[rows, :]
                if t < 8:
                    return a0_r0[rows, 128 * (t - 1) : 128 * t]
                return a0_r1[rows, 128 * (t - 8) : 128 * (t - 7)]

            def rhs_ap(p, n0, rows):
                if p == 1:
                    return b1_sb[rows, n0 : n0 + 512]
                return b0_q[n0 // 512][rows, :]

            dve_t = act_t = 0.0
            for p in range(BPC // 2):       # batch pair
                for t in range(MT):         # m tile
                    lhs0 = lhs_ap(p, t, slice(0, 64))
                    lhs1 = lhs_ap(p, t, slice(64, 128))
                    o0 = out_pool.tile([128, N], i8, tag="o")
                    o1 = out_pool.tile([128, N], i8, tag="o")
                    for h in range(NHALF):  # 1024-col drain unit
                        ps0 = psum_pool.tile([128, UNIT], f32, tag="ps")
                        ps1 = psum_pool.tile([128, UNIT], f32, tag="ps")
                        for j in range(2):  # 512-col matmul within unit
                            n0 = UNIT * h + 512 * j
                            c = slice(512 * j, 512 * (j + 1))
                            nc.tensor.matmul(
                                ps0[:, c],
                                lhs0,
                                rhs_ap(p, n0, slice(0, 64)),
                                start=True,
                                stop=True,
                            )
                            nc.tensor.matmul(
                                ps1[:, c],
                                lhs1,
                                rhs_ap(p, n0, slice(64, 128)),
                                start=True,
                                stop=True,
                            )
                        hs = slice(UNIT * h, UNIT * (h + 1))
                        for o, ps in ((o0, ps0), (o1, ps1)):
                            if dve_t + DVE_NS <= act_t + ACT_NS:
                                nc.vector.tensor_scalar_mul(o[:, hs], ps[:, :], alpha)
                                dve_t += DVE_NS
                            else:
                                nc.scalar.activation(
                                    o[:, hs],
                                    ps[:, :],
                                    mybir.ActivationFunctionType.Copy,
                                    scale=alpha,
                                )
                                act_t += ACT_NS
                    for which, o in ((0, o0), (1, o1)):
                        nc.sync.dma_start(
                            out=out[2 * p + which, 128 * t : 128 * (t + 1), :],
                            in_=o[:, :],
                        )
    nc.compile()
    return nc


def kernel(a: np.ndarray, b: np.ndarray, alpha) -> np.ndarray:
    from concourse.bass_utils import run_bass_kernel_spmd

    a = np.asarray(a)
    b = np.asarray(b)
    alpha_f = float(np.asarray(alpha))

    key = alpha_f
    if key not in _CACHE:
        _CACHE[key] = _build(alpha_f)
    nc = _CACHE[key]

    # host-side layout prep: per batch, [seq, K] int8 -> [K, seq] bf16, then
    # stack batch pairs along the partition axis.
    aT = np.ascontiguousarray(a.transpose(0, 2, 1)).astype(ml_dtypes.bfloat16)
    bT = np.ascontiguousarray(b.transpose(0, 2, 1)).astype(ml_dtypes.bfloat16)
    aT = aT.reshape(NCORES, BPC // 2, 128, M)
    bT = bT.reshape(NCORES, BPC // 2, 128, N)

    in_maps = [{"aT": aT[c], "bT": bT[c]} for c in range(NCORES)]
    res = run_bass_kernel_spmd(nc, in_maps, core_ids=list(range(NCORES)))
    outs = [res.results[c]["out"] for c in range(NCORES)]
    return np.concatenate(outs, axis=0).astype(np.int8)


# revision 23
# speedup vs baseline: 1.0144x; 1.0144x over previous
"""Bass/TRN2 kernel for nn_BMM_S8T_S8N_S8T:
    out[b,m,n] = sat_i8(round(alpha * sum_k a[b,m,k] * b[b,n,k]))
with a: (32, 2048, 64) int8, b: (32, 2048, 64) int8, alpha: f32 scalar.

Sharding: batch dim 32 -> 8 cores x 4 batches (pure data parallel, no
cross-core communication).

Per-core design notes:
  - int8 matmul is not supported by the PE; bf16 x bf16 -> f32 PSUM is exact
    for int8 operands (products < 2^16, sums of 64 < 2^24), so inputs are
    converted to bf16 (and pre-transposed to [K, seq] layout) on host.
  - The 4 local batches are stacked in pairs along SBUF partitions:
    partitions 0-63 hold batch 2p's K=64, partitions 64-127 batch 2p+1's.
    Row-tiled matmuls (tile_position row groups 0 and 2) then run the two
    batches' K=64 contractions concurrently on the 128x128 PE array.
  - Requant drain (PSUM f32 -> SBUF int8, scale + round-half-even + saturate)
    is the bottleneck: only VectorE and ScalarE can read PSUM, at ~1 elem/
    lane/cycle. Both engines drain in parallel on different PSUM banks, in
    [128, 1024] (2-bank) units so fills overlap drains within 8 banks.
    A single tensor_scalar_mul / activation(Copy, scale) instruction does the
    whole requant bit-exactly (verified vs numpy round/clip on HW).
"""

import numpy as np
import ml_dtypes

B, M, N, K = 32, 2048, 2048, 64
NCORES = 8
BPC = B // NCORES          # batches per core (4)
MT = M // 128              # m-tiles per batch (16)
NHALF = 2                  # two 1024-col drain units per m-tile row block
UNIT = N // NHALF          # 1024 columns per drain unit

_CACHE = {}


def _build(alpha: float):
    import concourse.bacc as bacc
    import concourse.mybir as mybir
    from concourse.tile import TileContext

    bf16 = mybir.dt.bfloat16
    f32 = mybir.dt.float32
    i8 = mybir.dt.int8

    nc = bacc.Bacc("TRN2")
    aT = nc.dram_tensor("aT", (BPC // 2, 128, M), bf16, kind="ExternalInput")
    bT = nc.dram_tensor("bT", (BPC // 2, 128, N), bf16, kind="ExternalInput")
    out = nc.dram_tensor("out", (BPC, M, N), i8, kind="ExternalOutput")

    # engine load balancing between the two drain engines (ns per [128,1024]
    # unit, hardware-measured)
    DVE_NS, ACT_NS = 1224.0, 1113.0
    WARMUP_MM = 8  # ~4.5us of dummy matmuls to lift the PE HAM clock gate

    # scratch operand for warm-up matmuls: deliberately never written (the
    # values don't matter and the scratch PSUM bank is never read); a raw
    # (non-pool) tensor so Tile's release tracking doesn't object.
    wz = nc.alloc_sbuf_tensor("wz", [128, 512], bf16)

    with TileContext(nc) as tc:
        with (
            tc.tile_pool(name="inp", bufs=1) as inp_pool,
            tc.tile_pool(name="ps", bufs=4, space="PSUM") as psum_pool,
            tc.tile_pool(name="outp", bufs=6) as out_pool,
        ):
            wps = psum_pool.tile([128, UNIT], f32, tag="ps")
            for _ in range(WARMUP_MM):
                nc.tensor.matmul(
                    wps[:, 0:512], wz[:, 0:128], wz[:, 0:512], start=True, stop=True
                )

            # Input tiles are split by "when first needed" — dependencies are
            # tile-granular and each individual DMA transfer only streams at
            # ~90-100 GB/s, so pair-0 inputs are loaded as several concurrent
            # DMAs (across the Sync HWDGE and GpSimd SWDGE queue streams) in
            # order of first use, letting the t=0 matmuls start early while
            # the rest lands behind the already-running pipeline.
            b0_q = [
                inp_pool.tile([128, 512], bf16, tag=f"b0q{q}", name=f"b0q{q}")
                for q in range(4)
            ]
            a0_first = inp_pool.tile([128, 128], bf16, tag="a0f")
            a0_r0 = inp_pool.tile([128, 896], bf16, tag="a0r0")
            a0_r1 = inp_pool.tile([128, 1024], bf16, tag="a0r1")
            a1_sb = inp_pool.tile([128, M], bf16, tag="a1")
            b1_sb = inp_pool.tile([128, N], bf16, tag="b1")
            # completions are FIFO per DGE ring, so the b pieces stream on the
            # Sync HWDGE ring and the a pieces on the GpSimd SWDGE ring, each
            # in order of first use
            for q in range(4):
                nc.sync.dma_start(
                    out=b0_q[q][:, :], in_=bT[0, :, 512 * q : 512 * (q + 1)]
                )
            nc.sync.dma_start(out=b1_sb[:, :], in_=bT[1])
            nc.gpsimd.dma_start(out=a0_first[:, :], in_=aT[0, :, 0:128])
            nc.gpsimd.dma_start(out=a0_r0[:, :], in_=aT[0, :, 128:1024])
            nc.gpsimd.dma_start(out=a0_r1[:, :], in_=aT[0, :, 1024:M])
            nc.gpsimd.dma_start(out=a1_sb[:, :], in_=aT[1])

            def lhs_ap(p, t, rows):
                if p == 1:
                    return a1_sb[rows, 128 * t : 128 * (t + 1)]
                if t == 0:
                    return a0_first[rows, :]
                if t < 8:
                    return a0_r0[rows, 128 * (t - 1) : 128 * t]
                return a0_r1[rows, 128 * (t - 8) : 128 * (t - 7)]

            def rhs_ap(p, n0, rows):
                if p == 1:
                    return b1_sb[rows, n0 : n0 + 512]
                return b0_q[n0 // 512][rows, :]

            dve_t = act_t = 0.0
            for p in range(BPC // 2):       # batch pair
                for t in range(MT):         # m tile
                    lhs0 = lhs_ap(p, t, slice(0, 64))
                    lhs1 = lhs_ap(p, t, slice(64, 128))
                    o0 = out_pool.tile([128, N], i8, tag="o")
                    o1 = out_pool.tile([128, N], i8, tag="o")
                    for h in range(NHALF):  # 1024-col drain unit
                        ps0 = psum_pool.tile([128, UNIT], f32, tag="ps")
                        ps1 = psum_pool.tile([128, UNIT], f32, tag="ps")
                        for j in range(2):  # 512-col matmul within unit
                            n0 = UNIT * h + 512 * j
                            c = slice(512 * j, 512 * (j + 1))
                            nc.tensor.matmul(
                                ps0[:, c],
                                lhs0,
                                rhs_ap(p, n0, slice(0, 64)),
                                start=True,
                                stop=True,
                            )
                            nc.tensor.matmul(
                                ps1[:, c],
                                lhs1,
                                rhs_ap(p, n0, slice(64, 128)),
                                start=True,
                                stop=True,
                            )
                        hs = slice(UNIT * h, UNIT * (h + 1))
                        for o, ps in ((o0, ps0), (o1, ps1)):
                            if dve_t + DVE_NS <= act_t + ACT_NS:
                                nc.vector.tensor_scalar_mul(o[:, hs], ps[:, :], alpha)
                                dve_t += DVE_NS
                            else:
                                nc.scalar.activation(
                                    o[:, hs],
                                    ps[:, :],
                                    mybir.ActivationFunctionType.Copy,
                                    scale=alpha,
                                )
                                act_t += ACT_NS
                    for which, o in ((0, o0), (1, o1)):
                        nc.sync.dma_start(
                            out=out[2 * p + which, 128 * t : 128 * (t + 1), :],
                            in_=o[:, :],
                        )
    nc.compile()
    return nc


def kernel(a: np.ndarray, b: np.ndarray, alpha) -> np.ndarray:
    from concourse.bass_utils import run_bass_kernel_spmd

    a = np.asarray(a)
    b = np.asarray(b)
    alpha_f = float(np.asarray(alpha))

    key = alpha_f
    if key not in _CACHE:
        _CACHE[key] = _build(alpha_f)
    nc = _CACHE[key]

    # host-side layout prep: per batch, [seq, K] int8 -> [K, seq] bf16, then
    # stack batch pairs along the partition axis.
    aT = np.ascontiguousarray(a.transpose(0, 2, 1)).astype(ml_dtypes.bfloat16)
    bT = np.ascontiguousarray(b.transpose(0, 2, 1)).astype(ml_dtypes.bfloat16)
    aT = aT.reshape(NCORES, BPC // 2, 128, M)
    bT = bT.reshape(NCORES, BPC // 2, 128, N)

    in_maps = [{"aT": aT[c], "bT": bT[c]} for c in range(NCORES)]
    res = run_bass_kernel_spmd(nc, in_maps, core_ids=list(range(NCORES)))
    outs = [res.results[c]["out"] for c in range(NCORES)]
    return np.concatenate(outs, axis=0).astype(np.int8)


# revision 24
# speedup vs baseline: 1.0387x; 1.0240x over previous
"""Bass/TRN2 kernel for nn_BMM_S8T_S8N_S8T:
    out[b,m,n] = sat_i8(round(alpha * sum_k a[b,m,k] * b[b,n,k]))
with a: (32, 2048, 64) int8, b: (32, 2048, 64) int8, alpha: f32 scalar.

Sharding: batch dim 32 -> 8 cores x 4 batches (pure data parallel, no
cross-core communication).

Per-core design notes:
  - int8 matmul is not supported by the PE; bf16 x bf16 -> f32 PSUM is exact
    for int8 operands (products < 2^16, sums of 64 < 2^24), so inputs are
    converted to bf16 (and pre-transposed to [K, seq] layout) on host.
  - The 4 local batches are stacked in pairs along SBUF partitions:
    partitions 0-63 hold batch 2p's K=64, partitions 64-127 batch 2p+1's.
    Row-tiled matmuls (tile_position row groups 0 and 2) then run the two
    batches' K=64 contractions concurrently on the 128x128 PE array.
  - Requant drain (PSUM f32 -> SBUF int8, scale + round-half-even + saturate)
    is the bottleneck: only VectorE and ScalarE can read PSUM, at ~1 elem/
    lane/cycle. Both engines drain in parallel on different PSUM banks, in
    [128, 1024] (2-bank) units so fills overlap drains within 8 banks.
    A single tensor_scalar_mul / activation(Copy, scale) instruction does the
    whole requant bit-exactly (verified vs numpy round/clip on HW).
"""

import numpy as np
import ml_dtypes

B, M, N, K = 32, 2048, 2048, 64
NCORES = 8
BPC = B // NCORES          # batches per core (4)
MT = M // 128              # m-tiles per batch (16)
NHALF = 2                  # two 1024-col drain units per m-tile row block
UNIT = N // NHALF          # 1024 columns per drain unit

_CACHE = {}


def _build(alpha: float):
    import concourse.bacc as bacc
    import concourse.mybir as mybir
    from concourse.tile import TileContext

    bf16 = mybir.dt.bfloat16
    f32 = mybir.dt.float32
    i8 = mybir.dt.int8

    nc = bacc.Bacc("TRN2")
    aT = nc.dram_tensor("aT", (BPC // 2, 128, M), bf16, kind="ExternalInput")
    bT = nc.dram_tensor("bT", (BPC // 2, 128, N), bf16, kind="ExternalInput")
    out = nc.dram_tensor("out", (BPC, M, N), i8, kind="ExternalOutput")

    # engine load balancing between the two drain engines (ns per [128,1024]
    # unit, hardware-measured)
    DVE_NS, ACT_NS = 1224.0, 1113.0
    WARMUP_MM = 8  # ~4.5us of dummy matmuls to lift the PE HAM clock gate

    # scratch operand for warm-up matmuls: deliberately never written (the
    # values don't matter and the scratch PSUM bank is never read); a raw
    # (non-pool) tensor so Tile's release tracking doesn't object.
    wz = nc.alloc_sbuf_tensor("wz", [128, 512], bf16)

    with TileContext(nc) as tc:
        with (
            tc.tile_pool(name="inp", bufs=1) as inp_pool,
            tc.tile_pool(name="ps", bufs=4, space="PSUM") as psum_pool,
            tc.tile_pool(name="outp", bufs=6) as out_pool,
        ):
            wps = psum_pool.tile([128, UNIT], f32, tag="ps")
            for _ in range(WARMUP_MM):
                nc.tensor.matmul(
                    wps[:, 0:512], wz[:, 0:128], wz[:, 0:512], start=True, stop=True
                )

            # Input tiles are split by "when first needed" — dependencies are
            # tile-granular and each individual DMA transfer only streams at
            # ~90-100 GB/s, so pair-0 inputs are loaded as several concurrent
            # DMAs (across the Sync HWDGE and GpSimd SWDGE queue streams) in
            # order of first use, letting the t=0 matmuls start early while
            # the rest lands behind the already-running pipeline.
            b0_q = [
                inp_pool.tile([128, 512], bf16, tag=f"b0q{q}", name=f"b0q{q}")
                for q in range(4)
            ]
            a0_first = inp_pool.tile([128, 128], bf16, tag="a0f")
            a0_r0 = inp_pool.tile([128, 896], bf16, tag="a0r0")
            a0_r1 = inp_pool.tile([128, 1024], bf16, tag="a0r1")
            a1_sb = inp_pool.tile([128, M], bf16, tag="a1")
            b1_sb = inp_pool.tile([128, N], bf16, tag="b1")
            # completions are FIFO per DGE ring, so the b pieces stream on the
            # Sync HWDGE ring and the a pieces on the GpSimd SWDGE ring, each
            # in order of first use
            for q in range(4):
                nc.sync.dma_start(
                    out=b0_q[q][:, :], in_=bT[0, :, 512 * q : 512 * (q + 1)]
                )
            nc.sync.dma_start(out=b1_sb[:, :], in_=bT[1])
            nc.gpsimd.dma_start(out=a0_first[:, :], in_=aT[0, :, 0:128])
            nc.gpsimd.dma_start(out=a0_r0[:, :], in_=aT[0, :, 128:1024])
            nc.gpsimd.dma_start(out=a0_r1[:, :], in_=aT[0, :, 1024:M])
            nc.gpsimd.dma_start(out=a1_sb[:, :], in_=aT[1])

            def lhs_ap(p, t, rows):
                if p == 1:
                    return a1_sb[rows, 128 * t : 128 * (t + 1)]
                if t == 0:
                    return a0_first[rows, :]
                if t < 8:
                    return a0_r0[rows, 128 * (t - 1) : 128 * t]
                return a0_r1[rows, 128 * (t - 8) : 128 * (t - 7)]

            def rhs_ap(p, n0, rows):
                if p == 1:
                    return b1_sb[rows, n0 : n0 + 512]
                return b0_q[n0 // 512][rows, :]

            dve_t = act_t = 0.0
            for p in range(BPC // 2):       # batch pair
                for t in range(MT):         # m tile
                    lhs0 = lhs_ap(p, t, slice(0, 64))
                    lhs1 = lhs_ap(p, t, slice(64, 128))
                    o0 = out_pool.tile([128, N], i8, tag="o")
                    o1 = out_pool.tile([128, N], i8, tag="o")
                    for h in range(NHALF):  # 1024-col drain unit
                        ps0 = psum_pool.tile([128, UNIT], f32, tag="ps")
                        ps1 = psum_pool.tile([128, UNIT], f32, tag="ps")
                        for j in range(2):  # 512-col matmul within unit
                            n0 = UNIT * h + 512 * j
                            c = slice(512 * j, 512 * (j + 1))
                            nc.tensor.matmul(
                                ps0[:, c],
                                lhs0,
                                rhs_ap(p, n0, slice(0, 64)),
                                start=True,
                                stop=True,
                            )
                            nc.tensor.matmul(
                                ps1[:, c],
                                lhs1,
                                rhs_ap(p, n0, slice(64, 128)),
                                start=True,
                                stop=True,
                            )
                        hs = slice(UNIT * h, UNIT * (h + 1))
                        for o, ps in ((o0, ps0), (o1, ps1)):
                            if dve_t + DVE_NS <= act_t + ACT_NS:
                                nc.vector.tensor_scalar_mul(o[:, hs], ps[:, :], alpha)
                                dve_t += DVE_NS
                            else:
                                nc.scalar.activation(
                                    o[:, hs],
                                    ps[:, :],
                                    mybir.ActivationFunctionType.Copy,
                                    scale=alpha,
                                )
                                act_t += ACT_NS
                    for which, o in ((0, o0), (1, o1)):
                        nc.sync.dma_start(
                            out=out[2 * p + which, 128 * t : 128 * (t + 1), :],
                            in_=o[:, :],
                        )
    nc.compile()
    return nc


def kernel(a: np.ndarray, b: np.ndarray, alpha) -> np.ndarray:
    from concourse.bass_utils import run_bass_kernel_spmd

    a = np.asarray(a)
    b = np.asarray(b)
    alpha_f = float(np.asarray(alpha))

    key = alpha_f
    if key not in _CACHE:
        _CACHE[key] = _build(alpha_f)
    nc = _CACHE[key]

    # host-side layout prep: per batch, [seq, K] int8 -> [K, seq] bf16, then
    # stack batch pairs along the partition axis.
    aT = np.ascontiguousarray(a.transpose(0, 2, 1)).astype(ml_dtypes.bfloat16)
    bT = np.ascontiguousarray(b.transpose(0, 2, 1)).astype(ml_dtypes.bfloat16)
    aT = aT.reshape(NCORES, BPC // 2, 128, M)
    bT = bT.reshape(NCORES, BPC // 2, 128, N)

    in_maps = [{"aT": aT[c], "bT": bT[c]} for c in range(NCORES)]
    try:
        res = run_bass_kernel_spmd(nc, in_maps, core_ids=list(range(NCORES)))
    except Exception:
        # one retry in case a previous process left a device in a bad state
        res = run_bass_kernel_spmd(nc, in_maps, core_ids=list(range(NCORES)))
    outs = [res.results[c]["out"] for c in range(NCORES)]
    return np.concatenate(outs, axis=0).astype(np.int8)
